# revision 1
# baseline (speedup 1.0000x reference)
"""GAT encoder (2x GATConv + ELU + global mean pool) on 8 Trainium2 NeuronCores.

Self-contained: kernel(**inputs) takes the FULL inputs (as produced by the
problem's setup_inputs), shards the graph across 8 cores, compiles + runs a
Bass/Tile SPMD kernel via run_bass_kernel_spmd, and returns the FULL [64, 128]
output.

v2 structure (single-shot-latency focused):
- Nodes are assigned to cores round-robin by in-degree rank (balances the
  per-block max degrees across cores), then each core's nodes are split into
  two SLICES (table halves) and shell-sorted within each slice so the padded
  per-block edge-slot widths (ja/jb) are uniform across cores.
- Each AllGather is split into two slice-collectives: AG-a (slice 0) fires as
  soon as the first 25 producer blocks are done and overlaps the producer
  tail; the B-half gathers (which need AG-b) are emitted LAG blocks behind the
  A-half gathers so the GPSIMD engine never stalls on the second collective.
- P3 (layer-2 table build) is fused into the P2 block loop.
- Edge pipeline uses packed-bf16 DVE layouts (2x mode) and in-place tree
  reduction over edge slots instead of a strided flat reduce; the layer-1
  ex-broadcast expansion runs on the ACT engine.
- build_kernel(repeat=K) chains rep i+1's weights on rep i's output so the
  K-fold wall-time slope measures true single-invocation latency.
"""
import sys

for _p in ("/opt/trn_rl_repo", "/root/.axon_site/_ro/trn_rl_repo"):
    if _p not in sys.path:
        sys.path.insert(0, _p)

from dataclasses import dataclass, field

import numpy as np

import concourse.bacc as bacc
import concourse.mybir as mybir
import concourse.tile as tile
from concourse import library_config
from concourse.bass_utils import run_bass_kernel_spmd

AF = mybir.ActivationFunctionType
OP = mybir.AluOpType
F32 = mybir.dt.float32
I16 = mybir.dt.int16
BF16 = mybir.dt.bfloat16

# problem constants (hardcoded per contract)
N_NODES = 50000
N_CORES = 8
IN_CH = 128
HID = 32
HEADS = 2
OUT_CH = 128
N_GRAPHS = 64
NEG_SLOPE = 0.2

HALF = 3200                       # padded nodes per (core, slice)
NPP = 2 * HALF                    # 6400 padded nodes per core
NB = NPP // 128                   # 50 blocks per core
NBH = NB // 2                     # 25 blocks per slice
HG = N_CORES * HALF               # 25600 rows per table half (<= int16 max)
NG = 2 * HG                       # 51200 global padded ids
D1 = HEADS * HID                  # 64
D2 = OUT_CH                       # 128

ROW1 = 128  # bf16 elems per layer-1 table row (256B): [xl1 x64 | a_s1 f32 x2 | pad]
ROW2 = 128  # bf16 elems per layer-2 table row (256B): [xl2 x128]


@dataclass
class Prep:
    ja: list
    jb: list
    idxa_w: list = field(default_factory=list)
    idxb_w: list = field(default_factory=list)
    mask_u: list = field(default_factory=list)
    onehot: list = field(default_factory=list)
    counts: np.ndarray | None = None
    node_lists: np.ndarray | None = None  # [N_CORES, NPP] original ids, -1 pad


def _wrap_idxs(idxs: np.ndarray) -> np.ndarray:
    S = len(idxs)
    cols = S // 16
    a = idxs.astype(np.int16).reshape(cols, 16).T
    return np.tile(a, (8, 1)).copy()


def preprocess(edge_index: np.ndarray, batch: np.ndarray) -> Prep:
    src = np.concatenate([edge_index[0], np.arange(N_NODES, dtype=np.int64)])
    dst = np.concatenate([edge_index[1], np.arange(N_NODES, dtype=np.int64)])
    deg = np.bincount(dst, minlength=N_NODES)

    # round-robin by in-degree rank -> every core sees the same degree profile
    rank = np.argsort(-deg, kind="stable")
    core_of_node = np.empty(N_NODES, np.int64)
    pos0 = np.empty(N_NODES, np.int64)
    core_of_node[rank] = np.arange(N_NODES) % N_CORES
    pos0[rank] = np.arange(N_NODES) // N_CORES
    slice_of = (pos0 >= HALF).astype(np.int64)  # fixed before the shell sort

    degA = np.zeros(N_NODES, np.int64)
    degB = np.zeros(N_NODES, np.int64)
    sA = slice_of[src] == 0
    np.add.at(degA, dst[sA], 1)
    np.add.at(degB, dst[~sA], 1)

    # shell sort within each (core, slice): aligns per-block (degA, degB)
    pos = np.empty(N_NODES, np.int64)
    node_lists = np.full((N_CORES, NPP), -1, np.int64)
    for k in range(N_CORES):
        for s in (0, 1):
            sel = np.flatnonzero((core_of_node == k) & (slice_of == s))
            m = np.maximum(degA[sel], degB[sel])
            o = sel[np.lexsort((-(degA[sel] - degB[sel]), -m))]
            pos[o] = s * HALF + np.arange(len(o))
            node_lists[k, s * HALF : s * HALF + len(o)] = o

    gid = slice_of * HG + core_of_node * HALF + (pos - slice_of * HALF)

    # per-block slot widths, uniform across cores
    blk = pos // 128
    jA = np.zeros((N_CORES, NB), np.int64)
    jB = np.zeros((N_CORES, NB), np.int64)
    np.maximum.at(jA, (core_of_node, blk), degA)
    np.maximum.at(jB, (core_of_node, blk), degB)
    ja = np.maximum(jA.max(axis=0), 1)
    jb = np.maximum(jB.max(axis=0), 1)

    prep = Prep(ja=[int(v) for v in ja], jb=[int(v) for v in jb])
    prep.node_lists = node_lists
    prep.counts = np.bincount(batch, minlength=N_GRAPHS).astype(np.float32)

    # per-edge slot assignment
    e_core = core_of_node[dst]
    e_pos = pos[dst]
    e_half_a = slice_of[src] == 0
    e_gsrc = gid[src]
    key = (e_core * NPP + e_pos) * 2 + (~e_half_a)
    order_e = np.argsort(key, kind="stable")
    ksorted = key[order_e]
    firsts = np.r_[0, np.flatnonzero(np.diff(ksorted)) + 1]
    startrep = np.repeat(firsts, np.diff(np.r_[firsts, len(ksorted)]))
    jwithin = np.arange(len(ksorted)) - startrep
    e_j = np.empty_like(jwithin)
    e_j[order_e] = jwithin

    SA = int(ja.sum()) * 128
    SB = int(jb.sum()) * 128
    offA = np.r_[0, np.cumsum(ja)[:-1]] * 128
    offB = np.r_[0, np.cumsum(jb)[:-1]] * 128
    JTOT = int(ja.sum() + jb.sum())
    offM = np.r_[0, np.cumsum(ja + jb)[:-1]]

    for k in range(N_CORES):
        sel = e_core == k
        p_k = e_pos[sel]
        j_k = e_j[sel]
        a_k = e_half_a[sel]
        g_k = e_gsrc[sel]
        b_k = p_k // 128
        pp_k = p_k % 128

        idxA = np.zeros(SA, dtype=np.int64)
        idxB = np.zeros(SB, dtype=np.int64)
        mask = np.full((128, JTOT), -1e30, dtype=np.float32)

        sa = a_k
        iA = offA[b_k[sa]] + j_k[sa] * 128 + pp_k[sa]
        idxA[iA] = g_k[sa]
        mask[pp_k[sa], offM[b_k[sa]] + j_k[sa]] = 0.0
        sb_ = ~a_k
        iB = offB[b_k[sb_]] + j_k[sb_] * 128 + pp_k[sb_]
        idxB[iB] = g_k[sb_] - HG
        mask[pp_k[sb_], offM[b_k[sb_]] + ja[b_k[sb_]] + j_k[sb_]] = 0.0

        prep.idxa_w.append(_wrap_idxs(idxA))
        prep.idxb_w.append(_wrap_idxs(idxB))
        prep.mask_u.append(mask)

        oh = np.zeros((128, NB * N_GRAPHS), dtype=np.float32)
        nodes = node_lists[k]
        real = nodes >= 0
        ppos = np.arange(NPP)[real]
        bvals = batch[nodes[real]]
        oh[ppos % 128, (ppos // 128) * N_GRAPHS + bvals] = 1.0
        prep.onehot.append(oh)

    return prep


def fold_weights(W1, att_src1, att_dst1, W2, att_src2, att_dst2):
    W1e = np.zeros((IN_CH, D1 + 2 * HEADS), dtype=np.float32)
    W1e[:, :D1] = W1
    for h in range(HEADS):
        W1e[:, D1 + h] = W1[:, h * HID : (h + 1) * HID] @ att_src1[h]
        W1e[:, D1 + HEADS + h] = W1[:, h * HID : (h + 1) * HID] @ att_dst1[h]
    W2e = np.zeros((D1, D2 + 2), dtype=np.float32)
    W2e[:, :D2] = W2
    W2e[:, D2] = W2 @ att_src2[0]
    W2e[:, D2 + 1] = W2 @ att_dst2[0]
    return W1e, W2e


def _tree_reduce(nc, g_ap, J, width):
    """In-place tree sum over the slot axis j of g_ap[:, 0:J, 0:width].

    All adds are packed-bf16 slab adds (DVE 2x mode). Result in g_ap[:, 0, :].
    """
    n = J
    while n > 1:
        h_ = n // 2
        nc.vector.tensor_tensor(
            g_ap[:, 0:h_, 0:width], g_ap[:, 0:h_, 0:width],
            g_ap[:, n - h_ : n, 0:width], OP.add,
        )
        n = n - h_


def _tree_reduce_last(nc, q_ap, J, width):
    """In-place tree sum over the LAST axis of q_ap[:, 0:J, 0:width].

    width must be a power of two. Result in q_ap[:, :, 0].
    """
    w = width
    while w > 1:
        h_ = w // 2
        nc.vector.tensor_tensor(
            q_ap[:, 0:J, 0:h_], q_ap[:, 0:J, 0:h_], q_ap[:, 0:J, h_:w], OP.add,
        )
        w = h_


def _edge_half_l1(nc, pool, g, nj, mask_ap, ad_ap, lowp):
    """Edge math for layer 1 on gathered g [128, nj, ROW1] bf16.

    Row: [xl1 x64 bf16 | a_s1 packed f32 x2 | pad].
    Leaves the weighted message sum in g[:, 0, 0:64] (bf16) and returns the
    softmax denominator [128, H] f32.
    """
    H, DH = HEADS, HID
    a_s = g[:, :, D1 : D1 + 2 * H].bitcast(F32)  # [128, nj, 2] f32
    s = pool.tile([128, nj, H], F32, tag="s", name="s")
    for h in range(H):
        nc.vector.scalar_tensor_tensor(
            s[:, :, h], a_s[:, :, h], ad_ap[:, h : h + 1], mask_ap, OP.add, OP.add
        )
    lr = pool.tile([128, nj, H], F32, tag="lr", name="lr")
    nc.vector.scalar_tensor_tensor(lr[:, :, :], s[:, :, :], NEG_SLOPE, s[:, :, :], OP.mult, OP.max)
    ex = pool.tile([128, nj, H], F32, tag="ex", name="ex")
    denom = pool.tile([128, H], F32, tag="denom", name="denom")
    for h in range(H):
        nc.scalar.activation(ex[:, :, h], lr[:, :, h], AF.Exp, accum_out=denom[:, h : h + 1])
    # expand ex -> [128, nj, 64] bf16 on ACT so the DVE multiply stays packed
    exB = pool.tile([128, nj, H * DH], BF16, tag="exB", name="exB", bufs=2)
    nc.scalar.activation(
        exB[:, :, :].rearrange("p j (h c) -> p j h c", h=H),
        ex[:, :, :].unsqueeze(3).broadcast_to([128, nj, H, DH]), AF.Copy,
    )
    nc.vector.tensor_tensor(g[:, :, 0:D1], g[:, :, 0:D1], exB[:, :, :], OP.mult)
    with lowp():
        _tree_reduce(nc, g, nj, D1)
    return denom


def _edge_half_l2(nc, tpool, pool, g, nj, mask_ap, ad_ap, att2_sb, lowp):
    """Edge math for layer 2 on gathered g [128, nj, ROW2=128] bf16.

    Leaves the weighted message sum in g[:, 0, :] (bf16) and returns the
    softmax denominator [128, 1] f32.
    """
    q = tpool.tile([128, nj, D2], BF16, tag="tmp", name="tmp")
    nc.vector.tensor_tensor(
        q[:, :, :], g[:, :, :],
        att2_sb[:, :].unsqueeze(1).broadcast_to([128, nj, D2]), OP.mult,
    )
    a_s = pool.tile([128, nj], F32, tag="as2", name="as2")
    nc.vector.tensor_reduce(a_s[:, :], q[:, :, :], mybir.AxisListType.X, OP.add)
    s = pool.tile([128, nj], F32, tag="s2", name="s2")
    nc.vector.scalar_tensor_tensor(
        s[:, :], a_s[:, :], ad_ap[:, 0:1], mask_ap, OP.add, OP.add
    )
    lr = pool.tile([128, nj], F32, tag="lr2", name="lr2")
    nc.vector.scalar_tensor_tensor(lr[:, :], s[:, :], NEG_SLOPE, s[:, :], OP.mult, OP.max)
    ex = pool.tile([128, nj], F32, tag="ex2", name="ex2")
    denom = pool.tile([128, 1], F32, tag="denom2", name="denom2")
    nc.scalar.activation(ex[:, :], lr[:, :], AF.Exp, accum_out=denom[:, 0:1])
    nc.vector.tensor_tensor(
        g[:, :, :], g[:, :, :],
        ex[:, :].unsqueeze(2).broadcast_to([128, nj, D2]), OP.mult,
    )
    with lowp():
        _tree_reduce(nc, g, nj, D2)
    return denom


def _finalize(nc, pool, d, osum_f, denom_f, bias_sb, nheads, suff):
    """res = elu(osum / max(denom,eps) + bias) with per-head denominators."""
    DHl = d // nheads
    dcl = pool.tile([128, nheads], F32, tag=f"dcl{suff}", name=f"dcl{suff}")
    nc.vector.tensor_scalar(dcl[:], denom_f[:], 1e-30, None, OP.max)
    rden = pool.tile([128, nheads], F32, tag=f"rden{suff}", name=f"rden{suff}")
    nc.vector.reciprocal(rden[:], dcl[:])
    pre = pool.tile([128, d], F32, tag=f"pre{suff}", name=f"pre{suff}")
    for h in range(nheads):
        nc.vector.scalar_tensor_tensor(
            pre[:, h * DHl : (h + 1) * DHl], osum_f[:, h * DHl : (h + 1) * DHl],
            rden[:, h : h + 1], bias_sb[:, h * DHl : (h + 1) * DHl], OP.mult, OP.add,
        )
    e1 = pool.tile([128, d], F32, tag=f"e1{suff}", name=f"e1{suff}")
    nc.scalar.activation(e1[:], pre[:], AF.Exp)
    t2 = pool.tile([128, d], F32, tag=f"t2{suff}", name=f"t2{suff}")
    nc.vector.tensor_scalar(t2[:], e1[:], 1.0, 0.0, OP.subtract, OP.min)
    res = pool.tile([128, d], F32, tag=f"res{suff}", name=f"res{suff}")
    nc.vector.tensor_tensor(res[:], pre[:], t2[:], OP.max)
    return res


def build_kernel(ja: list, jb: list, nq: int = 4, repeat: int = 1, lag: int = 3,
                 chain: bool = True, gathers_on: bool = True, compute_on: bool = True,
                 nsplit: int = 3, sp: bool = False):
    C, H, G, IN = N_CORES, HEADS, N_GRAPHS, IN_CH
    SA = sum(ja) * 128
    SB = sum(jb) * 128
    JTOT = sum(ja) + sum(jb)
    offA = np.r_[0, np.cumsum(ja)[:-1]] * 128
    offB = np.r_[0, np.cumsum(jb)[:-1]] * 128
    offM = np.r_[0, np.cumsum(np.array(ja) + np.array(jb))[:-1]]

    nc = bacc.Bacc("TRN2", target_bir_lowering=False, debug=False, num_devices=C,
                   num_swdge_queues=nq)

    xT = nc.dram_tensor("xT", [IN, NPP], F32, kind="ExternalInput")
    w1e = nc.dram_tensor("w1e", [IN, D1 + 2 * H], F32, kind="ExternalInput")
    w2e = nc.dram_tensor("w2e", [D1, D2 + 2], F32, kind="ExternalInput")
    b1r = nc.dram_tensor("b1r", [128, D1], F32, kind="ExternalInput")
    b2r = nc.dram_tensor("b2r", [128, D2], F32, kind="ExternalInput")
    att2 = nc.dram_tensor("att2", [128, D2], F32, kind="ExternalInput")
    ident = nc.dram_tensor("ident", [128, 128], F32, kind="ExternalInput")
    idxa_d = nc.dram_tensor("idxa", [128, SA // 16], I16, kind="ExternalInput")
    idxb_d = nc.dram_tensor("idxb", [128, SB // 16], I16, kind="ExternalInput")
    mask_d = nc.dram_tensor("mask", [128, JTOT], F32, kind="ExternalInput")
    oneh_d = nc.dram_tensor("onehot", [128, NB * G], F32, kind="ExternalInput")
    out_d = nc.dram_tensor("pool_out", [G, D2], F32, kind="ExternalOutput")

    import contextlib

    def lowp():
        return nc.allow_low_precision(reason="bf16 slot-tree accumulation")

    with tile.TileContext(nc) as tc:
        nc.gpsimd.load_library(library_config.mlp)
        with (
            tc.tile_pool(name="const", bufs=1) as cpool,
            tc.tile_pool(name="gtiles", bufs=5) as gpool,
            tc.tile_pool(name="ttiles", bufs=2) as tpool,
            tc.tile_pool(name="work", bufs=3) as pool,
            tc.tile_pool(name="psum", bufs=2, space="PSUM") as psum,
            tc.tile_pool(name="poolacc", bufs=1, space="PSUM") as psacc,
            tc.tile_pool(name="dram", bufs=1, space="DRAM") as dpool,
        ):
            w1e_sb = cpool.tile([IN, D1 + 2 * H], F32)
            w2e_f32 = cpool.tile([D1, D2 + 2], F32)
            w2e_sb = cpool.tile([D1, D2 + 2], BF16)
            b1_sb = cpool.tile([128, D1], F32)
            b2_sb = cpool.tile([128, D2], F32)
            att2_sb = cpool.tile([128, D2], BF16)
            att2_f32 = cpool.tile([128, D2], F32)
            id_sb = cpool.tile([128, 128], F32)

            for t, srcd in [
                (w1e_sb, w1e), (w2e_f32, w2e), (b1_sb, b1r), (b2_sb, b2r),
                (att2_f32, att2), (id_sb, ident),
            ]:
                nc.sync.dma_start(t[:], srcd[:])
            nc.vector.tensor_copy(att2_sb[:], att2_f32[:])
            nc.vector.tensor_copy(w2e_sb[:], w2e_f32[:])

            prev_out = None
            tables1, tables2 = [], []
            for _rep in range(repeat):
                # chain rep on previous output so reps measure true latency
                w1e_work = cpool.tile([IN, D1 + 2 * H], F32, name="w1e_work")
                if prev_out is None or not chain:
                    nc.vector.tensor_copy(w1e_work[:], w1e_sb[:])
                else:
                    nc.vector.scalar_tensor_tensor(
                        w1e_work[0:G, :], prev_out[0:G, 0 : D1 + 2 * H], 0.0,
                        w1e_sb[0:G, :], OP.mult, OP.add,
                    )
                    nc.vector.tensor_copy(w1e_work[G:IN, :], w1e_sb[G:IN, :])

                ad1_sb = cpool.tile([128, NB, H], F32, name="ad1_sb")
                ad2_sb = cpool.tile([128, NB], F32, name="ad2_sb")
                hT_sb = cpool.tile([D1, NPP], BF16, name="hT_sb")

                # P1: layer-1 table; AG1a fires after the first NBH blocks
                xl1_own = dpool.tile([NPP, ROW1], BF16, name="xl1_own")
                t1name = "table1_sh_0" if chain else f"table1_sh_{_rep}"
                if _rep == 0 or not chain:
                    _tbl1 = nc.dram_tensor(
                        t1name, [NG, ROW1], BF16,
                        kind="Internal", addr_space="Shared",
                    )
                    tables1.append(_tbl1)
                table1 = tables1[-1].ap()
                for c in range(NB):
                    xt_t = pool.tile([IN, 128], F32, tag="xt", name="xt")
                    nc.sync.dma_start(xt_t[:], xT[:, c * 128 : (c + 1) * 128])
                    ps = psum.tile([128, D1 + 2 * H], F32, tag="mm1", name="mm1")
                    nc.tensor.matmul(ps[:], xt_t[:], w1e_work[:], start=True, stop=True)
                    rowt = pool.tile([128, ROW1], BF16, tag="row1", name="row1")
                    nc.vector.memset(rowt[:, D1 + 2 * H : ROW1], 0.0)
                    nc.vector.tensor_copy(rowt[:, 0:D1], ps[:, 0:D1])
                    nc.vector.tensor_copy(
                        rowt[:, D1 : D1 + 2 * H].bitcast(F32), ps[:, D1 : D1 + H]
                    )
                    nc.sync.dma_start(xl1_own[c * 128 : (c + 1) * 128, :], rowt[:])
                    nc.vector.tensor_copy(ad1_sb[:, c, :], ps[:, D1 + H : D1 + 2 * H])
                    if c == NBH - 1:
                        nc.gpsimd.collective_compute(
                            "AllGather", OP.bypass, replica_groups=[list(range(C))],
                            ins=[xl1_own[0:HALF, :].opt()], outs=[table1[0:HG, :]],
                        )
                nc.gpsimd.collective_compute(
                    "AllGather", OP.bypass, replica_groups=[list(range(C))],
                    ins=[xl1_own[HALF:NPP, :].opt()], outs=[table1[HG:NG, :]],
                )

                xl2_own = dpool.tile([NPP, ROW2], BF16, name="xl2_own")
                if _rep == 0 or not chain:
                    _tbl2 = nc.dram_tensor(
                        "table2_sh_0" if chain else f"table2_sh_{_rep}", [NG, ROW2], BF16,
                        kind="Internal", addr_space="Shared",
                    )
                    tables2.append(_tbl2)
                table2 = tables2[-1].ap()

                def gathers(b, table, g):
                    """Both half-gathers for block b into one g tile.

                    Each half is split into nsplit gather instructions fanned
                    across the 4 SWDGE queues (random 256B reads need the ring
                    parallelism).
                    """
                    idxa_t = pool.tile([128, 8 * ja[b]], I16, tag="idxa", name="idxa")
                    nc.sync.dma_start(
                        idxa_t[:], idxa_d[:, offA[b] // 16 : (offA[b] + 128 * ja[b]) // 16]
                    )
                    idxb_t = pool.tile([128, 8 * jb[b]], I16, tag="idxb", name="idxb")
                    nc.sync.dma_start(
                        idxb_t[:], idxb_d[:, offB[b] // 16 : (offB[b] + 128 * jb[b]) // 16]
                    )
                    parts = []
                    for (w, tab, it, base) in [
                        (ja[b], table[0:HG, :], idxa_t, 0),
                        (jb[b], table[HG:NG, :], idxb_t, ja[b]),
                    ]:
                        cuts = [w * i // nsplit for i in range(nsplit + 1)]
                        for c0, c1 in zip(cuts[:-1], cuts[1:]):
                            if c1 > c0:
                                parts.append((base, c0, c1, tab, it))
                    qa = (2 * b) % nq
                    for i, (base, c0, c1, tab, it) in enumerate(parts):
                        nj = c1 - c0
                        nc.gpsimd.dma_gather(
                            g[:, base + c0 : base + c1, :], tab,
                            it[:, 8 * c0 : 8 * c1],
                            nj * 128, nj * 128, ROW1,
                            single_packet=sp, queue_num=(qa + i) % nq,
                        )

                # P2 + fused P3, B-gathers implicit in the single g tile
                g_tiles = {}
                for i in range(NB + lag):
                    if i < NB:
                        b = i
                        J = ja[b] + jb[b]
                        g = gpool.tile([128, J, ROW1], BF16, tag="g", name="g")
                        g_tiles[b] = g
                        if gathers_on:
                            gathers(b, table1, g)
                    if i >= lag:
                        b = i - lag
                        J = ja[b] + jb[b]
                        g = g_tiles.pop(b)
                        if not compute_on:
                            cons = pool.tile([128, 2], BF16, tag="cons", name="cons")
                            nc.vector.tensor_copy(cons[:, :], g[:, 0, 0:2])
                            continue
                        mask_t = pool.tile([128, J], F32, tag="mask", name="mask")
                        nc.sync.dma_start(mask_t[:], mask_d[:, offM[b] : offM[b] + J])
                        den = _edge_half_l1(nc, pool, g, J, mask_t[:, :],
                                            ad1_sb[:, b, :], lowp)
                        res = _finalize(nc, pool, D1, g[:, 0, 0:D1], den, b1_sb, H, "1")
                        pst = psum.tile([D1, 128], F32, tag="tps", name="tps")
                        nc.tensor.transpose(pst[:], res[:], id_sb[:])
                        nc.vector.tensor_copy(hT_sb[:, b * 128 : (b + 1) * 128], pst[:])
                        ps2 = psum.tile([128, D2 + 2], F32, tag="mm2", name="mm2")
                        nc.tensor.matmul(
                            ps2[:], hT_sb[:, b * 128 : (b + 1) * 128], w2e_sb[:],
                            start=True, stop=True,
                        )
                        rowt2 = pool.tile([128, ROW2], BF16, tag="row2", name="row2")
                        nc.vector.tensor_copy(rowt2[:, 0:D2], ps2[:, 0:D2])
                        nc.sync.dma_start(xl2_own[b * 128 : (b + 1) * 128, :], rowt2[:])
                        nc.vector.tensor_copy(ad2_sb[:, b : b + 1], ps2[:, D2 + 1 : D2 + 2])
                        if b == NBH - 1:
                            nc.gpsimd.collective_compute(
                                "AllGather", OP.bypass, replica_groups=[list(range(C))],
                                ins=[xl2_own[0:HALF, :].opt()], outs=[table2[0:HG, :]],
                            )
                nc.gpsimd.collective_compute(
                    "AllGather", OP.bypass, replica_groups=[list(range(C))],
                    ins=[xl2_own[HALF:NPP, :].opt()], outs=[table2[HG:NG, :]],
                )
                # P4: layer-2 edges + pool
                pacc = psacc.tile([G, D2], F32, name="pacc")
                g_tiles = {}
                for i in range(NB + lag):
                    if i < NB:
                        b = i
                        J = ja[b] + jb[b]
                        g = gpool.tile([128, J, ROW2], BF16, tag="g", name="g")
                        g_tiles[b] = g
                        if gathers_on:
                            gathers(b, table2, g)
                    if i >= lag:
                        b = i - lag
                        J = ja[b] + jb[b]
                        g = g_tiles.pop(b)
                        if not compute_on:
                            cons = pool.tile([128, 2], BF16, tag="cons", name="cons")
                            nc.vector.tensor_copy(cons[:, :], g[:, 0, 0:2])
                            continue
                        mask_t = pool.tile([128, J], F32, tag="mask", name="mask")
                        nc.sync.dma_start(mask_t[:], mask_d[:, offM[b] : offM[b] + J])
                        den = _edge_half_l2(nc, tpool, pool, g, J, mask_t[:, :],
                                            ad2_sb[:, b : b + 1], att2_sb, lowp)
                        res = _finalize(nc, pool, D2, g[:, 0, :], den, b2_sb, 1, "2")
                        oh_t = pool.tile([128, G], F32, tag="oh", name="oh")
                        nc.sync.dma_start(oh_t[:], oneh_d[:, b * G : (b + 1) * G])
                        nc.tensor.matmul(
                            pacc[:], oh_t[:], res[:],
                            start=(b == 0), stop=(b == NB - 1),
                        )
                out_sb = pool.tile([G, D2], F32, tag="outsb", name="outsb")
                if compute_on:
                    nc.vector.tensor_copy(out_sb[:], pacc[:])
                else:
                    nc.vector.memset(out_sb[:], 0.0)
                nc.sync.dma_start(out_d[:], out_sb[:])
                prev_out = out_sb

    nc.compile()
    return nc


def _make_in_maps(x, W1e, W2e, b1, b2, as1f, as2f, prep: Prep):
    ident = np.eye(128, dtype=np.float32)
    b1r = np.tile(np.asarray(b1, np.float32).reshape(1, -1), (128, 1))
    b2r = np.tile(np.asarray(b2, np.float32).reshape(1, -1), (128, 1))
    att2 = np.tile(as2f.reshape(1, -1).astype(np.float32), (128, 1))
    in_maps = []
    for k in range(N_CORES):
        nodes = prep.node_lists[k]
        real = nodes >= 0
        xk = np.zeros((IN_CH, NPP), dtype=np.float32)
        xk[:, real] = x[nodes[real]].T
        in_maps.append(
            {
                "xT": xk, "w1e": W1e, "w2e": W2e, "b1r": b1r, "b2r": b2r,
                "att2": att2, "ident": ident,
                "idxa": prep.idxa_w[k], "idxb": prep.idxb_w[k],
                "mask": prep.mask_u[k], "onehot": prep.onehot[k],
            }
        )
    return in_maps


_CACHE = {}


def kernel(x, edge_index, batch, W1, att_src1, att_dst1, b1, W2, att_src2, att_dst2, b2):
    x = np.asarray(x, dtype=np.float32)
    edge_index = np.asarray(edge_index, dtype=np.int64)
    batch = np.asarray(batch, dtype=np.int64)
    W1 = np.asarray(W1, dtype=np.float32)
    W2 = np.asarray(W2, dtype=np.float32)
    att_src1 = np.asarray(att_src1, dtype=np.float32)
    att_dst1 = np.asarray(att_dst1, dtype=np.float32)
    att_src2 = np.asarray(att_src2, dtype=np.float32)
    att_dst2 = np.asarray(att_dst2, dtype=np.float32)

    prep = preprocess(edge_index, batch)
    W1e, W2e = fold_weights(W1, att_src1, att_dst1, W2, att_src2, att_dst2)

    key = (tuple(prep.ja), tuple(prep.jb))
    if key not in _CACHE:
        _CACHE[key] = build_kernel(prep.ja, prep.jb)
    nc = _CACHE[key]

    in_maps = _make_in_maps(
        x, W1e, W2e, b1, b2, att_src1.reshape(-1), att_src2.reshape(-1), prep
    )
    res = None
    last_err = None
    for attempt in range(4):
        try:
            res = run_bass_kernel_spmd(
                nc, in_maps, core_ids=list(range(N_CORES)), trace=False
            )
            break
        except Exception as e:  # device flake: reset backends and retry
            last_err = e
            import time as _time

            _time.sleep(8.0 * (attempt + 1))
            try:
                import jax as _jax

                _jax.clear_caches()
                _jax.extend.backend.clear_backends()
            except Exception:
                pass
    if res is None:
        raise last_err

    total = np.zeros((N_GRAPHS, OUT_CH), np.float64)
    for k in range(N_CORES):
        total += res.results[k]["pool_out"]
    out = total / np.maximum(prep.counts, 1.0)[:, None]
    return out.astype(np.float32)

